# revision 1
# baseline (speedup 1.0000x reference)
"""Trainium2 Bass kernel for single-head attention (no V projection).

Reference computation (per batch b):
    qk   = x @ W_qk.T + b_qk          # [n, 2d]
    q, k = qk[:, :d], qk[:, d:]
    dots[i, j] = k_i . q_j / sqrt(d)
    attn = softmax(dots, axis=-1)
    out[i] = sum_j attn[i, j] * x[j]

Sharding: data-parallel over batch b (8 batches -> 8 NeuronCores), no
collectives.  Per core, in "Q'=k, K'=q, V=x" standard-attention form:

  A:  QKh^T[dd, n] = W^T-chunks^T @ X^T  (PE, fp32r)  -> Kh^T (q-proj, SBUF
      resident) and Qh^T (k-proj, spilled to a DRAM scratch).
  B:  S^T[j, i] = Kh^T(:,j)^T @ Qh^T(:,i); E^T = exp(S^T / 32) (ACT).
  C:  out[i, :] = E^T(:,i)^T @ X accumulated over j in PSUM; the softmax
      denominator comes from a DVE running sum of the E^T strips followed by
      one tiny ones-matmul per 128-row block; normalize via per-partition
      reciprocal (DVE).

exp() is computed without max-subtraction: scores are ~N(0, 0.67) after the
1/sqrt(d) scale, so exp never overflows and softmax(x) == exp(x)/sum(exp(x)).

All matmul operands use float32r (tf32-class precision at full PE rate).

Host-side input layouts (computed in kernel()):
  w4 [128, 16*8*128]: w4[p, ((m*8)+k)*128+c] = W_qk[m*128+c, k*128+p]
      -> one [128, 1024] DMA per output row-block m covers all 8 k-chunks.
  bq4 [128, 16]:      bq4[p, m] = b_qk[m*128+p]
  xT [1024, 2048], xn [2048, 1024], ones [128, 8].
"""
import sys

try:
    import concourse.bass as bass  # noqa: F401
except ImportError:  # pragma: no cover
    sys.path.insert(0, "/opt/trn_rl_repo")

import numpy as np
import concourse.bass as bass
import concourse.mybir as mybir
import concourse.tile as tile
from concourse import bacc
from concourse.bass_utils import run_bass_kernel_spmd
import concourse.bass_utils as _bu

# Let walrus elide redundant LDWEIGHTS between consecutive matmuls that share
# the same stationary operand (stage C issues such pairs back-to-back).
if not getattr(_bu, "_ldw_opt_patched", False):
    _orig_run_command = _bu.run_command

    def _run_command_ldw(argv, **kwargs):
        argv = [a.replace("--enable-ldw-opt=false", "--enable-ldw-opt=true")
                if isinstance(a, str) else a for a in argv]
        return _orig_run_command(argv, **kwargs)

    _bu.run_command = _run_command_ldw
    _bu._ldw_opt_patched = True

B, N, D = 8, 2048, 1024
NCORES = 8
SCALE = 1.0 / np.sqrt(D)  # 1/32

_NC = None
LAST_RESULTS = None


def _build_nc():
    R = mybir.dt.float32r
    F = mybir.dt.float32
    nc = bacc.Bacc("TRN2", target_bir_lowering=False, debug=False, num_devices=NCORES)

    KD = D // 128        # 8 k-chunks over the projection contraction dim
    MB = 2 * D // 128    # 16 output row-blocks of the combined q/k projection
    NJ = N // 128        # 16 key blocks (j)
    CH = 512             # i-chunk width for stages B/C
    NCH = N // CH        # 4 chunks
    NSUB = CH // 128     # 4 row-subblocks per chunk

    xT = nc.dram_tensor("xT", [D, N], R, kind="ExternalInput").ap()
    xn = nc.dram_tensor("xn", [N, D], R, kind="ExternalInput").ap()
    w4 = nc.dram_tensor("w4", [128, MB * KD * 128], R, kind="ExternalInput").ap()
    bq4 = nc.dram_tensor("bq4", [128, MB], F, kind="ExternalInput").ap()
    ones = nc.dram_tensor("ones", [128, 8], R, kind="ExternalInput").ap()
    out = nc.dram_tensor("out", [N, D], F, kind="ExternalOutput").ap()

    with tile.TileContext(nc) as tc:
        with tc.tile_pool(name="kh", bufs=1) as khp, \
             tc.tile_pool(name="misc", bufs=1) as misc, \
             tc.tile_pool(name="ost", bufs=2) as ostp, \
             tc.tile_pool(name="qTd", bufs=1, space="DRAM") as qtp:

            kh = [khp.tile([128, N], R, tag=f"kh{k}", name=f"kh{k}") for k in range(KD)]
            # q4 scratch, viewed [128, KD, N]: row p holds Qh^T[k*128+p, :]
            q4 = qtp.tile([128, KD * N], R, tag="q4", name="q4")
            q4r = q4.rearrange("p (k i) -> p k i", k=KD)

            # ---------------- stage A: projections ----------------
            # n-chunk outer / m inner: the PE can start after one [128,1024]
            # weight DMA plus a single 2MB xT column-chunk instead of the
            # whole 8MB of xT.
            with tc.tile_pool(name="xt", bufs=1) as xtp, \
                 tc.tile_pool(name="wst", bufs=1) as wst, \
                 tc.tile_pool(name="qst", bufs=4) as qst, \
                 tc.tile_pool(name="psA", bufs=2, space="PSUM") as psA:
                morder = list(range(KD, MB)) + list(range(KD))
                xTr = xT.rearrange("(k p) i -> p k i", p=128)
                wtm = {}
                xtc = {}

                def load_xt_chunk(n):
                    xtc[n] = t = xtp.tile([128, KD * 512], R, tag=f"xtn{n % 2}",
                                          name=f"xt{n}")
                    nc.sync.dma_start(
                        out=t.rearrange("p (k i) -> p k i", k=KD),
                        in_=xTr[:, :, n * 512:(n + 1) * 512])

                # Split the very first weight/xT transfers so matmul (n=0,
                # m=morder[0], k=0) waits on only 64KB + 256KB of DMA.
                m0 = morder[0]
                wtm[m0] = wst.tile([128, KD * 128], R, tag=f"wtm{m0}",
                                   name=f"wtm{m0}")
                nc.sync.dma_start(out=wtm[m0][:, 0:128],
                                  in_=w4[:, m0 * KD * 128:m0 * KD * 128 + 128])
                xtc[0] = t0 = xtp.tile([128, KD * 512], R, tag="xtn0", name="xt0")
                t0r = t0.rearrange("p (k i) -> p k i", k=KD)
                nc.sync.dma_start(out=t0r[:, 0:1, :], in_=xTr[:, 0:1, 0:512])
                nc.sync.dma_start(out=wtm[m0][:, 128:KD * 128],
                                  in_=w4[:, m0 * KD * 128 + 128:(m0 + 1) * KD * 128])
                nc.sync.dma_start(out=t0r[:, 1:KD, :], in_=xTr[:, 1:KD, 0:512])
                m1 = morder[1]
                wtm[m1] = wst.tile([128, KD * 128], R, tag=f"wtm{m1}",
                                   name=f"wtm{m1}")
                nc.sync.dma_start(out=wtm[m1],
                                  in_=w4[:, m1 * KD * 128:(m1 + 1) * KD * 128])
                bias_all = misc.tile([128, MB], F, tag="bias", name="bias_all")
                nc.sync.dma_start(out=bias_all, in_=bq4)
                onesT = misc.tile([128, 8], R, tag="ones", name="onesT")
                nc.sync.dma_start(out=onesT, in_=ones)
                for m in morder[2:]:
                    wtm[m] = wst.tile([128, KD * 128], R, tag=f"wtm{m}",
                                      name=f"wtm{m}")
                    nc.sync.dma_start(out=wtm[m],
                                      in_=w4[:, m * KD * 128:(m + 1) * KD * 128])

                for n in range(4):
                    xt_n = xtc.pop(n)
                    if n + 1 < 4:
                        load_xt_chunk(n + 1)
                    cols = slice(n * 512, (n + 1) * 512)
                    for m in morder:
                        pt = psA.tile([128, 512], F, tag=f"a{m % 4}",
                                      name=f"psA{m % 4}")
                        for k in range(KD):
                            nc.tensor.matmul(
                                pt, wtm[m][:, k * 128:(k + 1) * 128],
                                xt_n[:, k * 512:(k + 1) * 512],
                                start=(k == 0), stop=(k == KD - 1))
                        if m < KD:
                            nc.vector.tensor_scalar_add(
                                kh[m][:, cols], pt, bias_all[:, m:m + 1])
                        else:
                            st = qst.tile([128, 512], R, tag="qs", name="qstage")
                            nc.vector.tensor_scalar_add(
                                st, pt, bias_all[:, m:m + 1])
                            nc.sync.dma_start(out=q4r[:, m - KD, cols], in_=st)

            # ---------------- stages B + C, fused per i-chunk ----------------
            with tc.tile_pool(name="xv", bufs=1) as xvp, \
                 tc.tile_pool(name="e", bufs=1) as ep, \
                 tc.tile_pool(name="es", bufs=1) as esp, \
                 tc.tile_pool(name="qc", bufs=2) as qcp, \
                 tc.tile_pool(name="psB", bufs=2, space="PSUM") as psB, \
                 tc.tile_pool(name="psO", bufs=2, space="PSUM") as psO:
                xv = [xvp.tile([128, D], R, tag=f"xv{j}", name=f"xv{j}")
                      for j in range(NJ)]
                for j in range(NJ):
                    nc.sync.dma_start(out=xv[j], in_=xn[j * 128:(j + 1) * 128, :])

                for c in range(NCH):
                    ccols = slice(c * CH, (c + 1) * CH)
                    # one DMA for all 8 k-chunks of Qh^T[:, chunk]
                    qc_all = qcp.tile([128, KD * CH], R, tag="qc", name="qc_all")
                    nc.sync.dma_start(
                        out=qc_all.rearrange("p (k i) -> p k i", k=KD),
                        in_=q4r[:, :, ccols])

                    es = []
                    esum = esp.tile([128, CH], F, tag="esum", name="esum")
                    for j in range(NJ):
                        ps = psB.tile([128, CH], F, tag="sB", name="psB")
                        for k in range(KD):
                            nc.tensor.matmul(
                                ps, kh[k][:, j * 128:(j + 1) * 128],
                                qc_all[:, k * CH:(k + 1) * CH],
                                start=(k == 0), stop=(k == KD - 1))
                        e_j = ep.tile([128, CH], R, tag=f"e{j}", name=f"e{j}")
                        nc.scalar.activation(
                            e_j, ps, mybir.ActivationFunctionType.Exp, scale=SCALE)
                        es.append(e_j)
                        # running fp32 sum over j-strips for the denominator
                        if j == 0:
                            nc.vector.tensor_copy(esum, e_j)
                        else:
                            nc.vector.tensor_add(esum, esum, e_j)
                    esumR = esp.tile([128, CH], R, tag="esumR", name="esumR")
                    nc.vector.tensor_copy(esumR, esum)

                    for sub in range(NSUB):
                        p0 = psO.tile([128, 512], F, tag="c0", name="psO0")
                        p1 = psO.tile([128, 512], F, tag="c1", name="psO1")
                        for j in range(NJ):
                            lhs = es[j][:, sub * 128:(sub + 1) * 128]
                            nc.tensor.matmul(p0, lhs, xv[j][:, 0:512],
                                             start=(j == 0), stop=(j == NJ - 1))
                            nc.tensor.matmul(p1, lhs, xv[j][:, 512:1024],
                                             start=(j == 0), stop=(j == NJ - 1))
                        pd = psO.tile([128, 8], F, tag="cd", name="psOd")
                        nc.tensor.matmul(pd, esumR[:, sub * 128:(sub + 1) * 128],
                                         onesT, start=True, stop=True)
                        rden = ostp.tile([128, 1], F, tag="rden", name="rden")
                        nc.vector.reciprocal(rden, pd[:, 0:1])
                        ob = ostp.tile([128, D], F, tag="ob", name="ob")
                        nc.vector.tensor_scalar_mul(ob[:, 0:512], p0, rden)
                        nc.vector.tensor_scalar_mul(ob[:, 512:1024], p1, rden)
                        row = c * CH + sub * 128
                        nc.sync.dma_start(out=out[row:row + 128, :], in_=ob)

    nc.finalize()
    return nc


def _get_nc():
    global _NC
    if _NC is None:
        _NC = _build_nc()
    return _NC


def _host_inputs(x_b, w4, bq4, ones):
    return {
        "xT": np.ascontiguousarray(x_b.T),
        "xn": np.ascontiguousarray(x_b),
        "w4": w4,
        "bq4": bq4,
        "ones": ones,
    }


def _prep_shared(W_qk, b_qk):
    W_qk = np.ascontiguousarray(W_qk, dtype=np.float32)
    # w4[p, m, k, c] = W_qk[m*128+c, k*128+p]
    w4 = np.ascontiguousarray(
        W_qk.reshape(16, 128, 8, 128).transpose(3, 0, 2, 1).reshape(128, -1))
    bq4 = np.ascontiguousarray(
        np.asarray(b_qk, dtype=np.float32).reshape(16, 128).T)
    ones = np.ones((128, 8), dtype=np.float32)
    return w4, bq4, ones


def kernel(x: np.ndarray, W_qk: np.ndarray, b_qk: np.ndarray) -> np.ndarray:
    global LAST_RESULTS
    assert x.shape == (B, N, D), x.shape
    nc = _get_nc()

    x = np.ascontiguousarray(x, dtype=np.float32)
    w4, bq4, ones = _prep_shared(W_qk, b_qk)
    in_maps = [_host_inputs(x[c], w4, bq4, ones) for c in range(NCORES)]

    res = run_bass_kernel_spmd(nc, in_maps, core_ids=list(range(NCORES)))
    LAST_RESULTS = res
    out = np.stack([res.results[c]["out"] for c in range(NCORES)], axis=0)
    return out.astype(np.float32)


if __name__ == "__main__":
    rng = np.random.default_rng(0)
    x = rng.standard_normal((B, N, D), dtype=np.float32)
    limit = float(np.sqrt(6.0 / (D + 2 * D)))
    W = rng.uniform(-limit, limit, size=(2 * D, D)).astype(np.float32)
    b = np.zeros((2 * D,), dtype=np.float32)
    got = kernel(x, W, b)
    print("out", got.shape, got.dtype)



# revision 8
# speedup vs baseline: 1.1899x; 1.1899x over previous
"""Trainium2 Bass kernel for single-head attention (no V projection).

Reference computation (per batch b):
    qk   = x @ W_qk.T + b_qk          # [n, 2d]
    q, k = qk[:, :d], qk[:, d:]
    dots[i, j] = k_i . q_j / sqrt(d)
    attn = softmax(dots, axis=-1)
    out[i] = sum_j attn[i, j] * x[j]

Key algebraic folding: dots[i,j] = (Wk x_i + bk).(Wq x_j + bq).  The terms
that depend only on i are constant along the softmax axis j and cancel, so
    softmax_j(dots[i,:]) == softmax_j(x_i^T A x_j + u.x_j),
with A = Wk^T Wq [d,d] and u = Wq^T bk, both input-independent and folded on
the host.  This removes the separate q/k projections: the device computes
    t   = X A + u                     # [n, d]   (stage A; u via ACT bias)
    S^T = X t^T                       # [n, n]   (stage B)
    E   = exp(S^T / 32)               # ACT, no max-subtraction (scores are
                                      #   ~N(0,0.67); exp cannot overflow)
    out = normalize(E^T X)            # (stage C; denominator via a DVE
                                      #   running sum + tiny ones-matmul)
cutting device matmul work from 3 N^2 d-class GEMMs + projections (1552
512-row matmuls) to 1296.

All matmul operands are bfloat16 (same PE rate as float32r on TRN2, half the
SBUF/DMA) — measured end-to-end error ~3e-3 scaled vs the 2e-2 gate.  Both x
layouts (xT for stages A/B, xn for stage C) and A stay SBUF-resident; there
is no DRAM spill.  Every stationary [128,128] block is loaded exactly once
(LDWEIGHTS is elided across the 2-4 matmuls sharing it and hides under the
matmul stream).

Sharding: data-parallel over batch b (8 batches -> 8 NeuronCores), no
collectives.
"""
import sys

try:
    import concourse.bass as bass  # noqa: F401
except ImportError:  # pragma: no cover
    sys.path.insert(0, "/opt/trn_rl_repo")

import numpy as np
import ml_dtypes
import concourse.bass as bass
import concourse.mybir as mybir
import concourse.tile as tile
from concourse import bacc
from concourse.bass_utils import run_bass_kernel_spmd
import concourse.bass_utils as _bu

# NOTE: walrus --enable-ldw-opt=true (used by the f32r baseline) is
# incompatible with the explicit InstLdweights that bacc synthesizes for
# multi-wait bf16 matmuls; keep the default (false).

B, N, D = 8, 2048, 1024
NCORES = 8
SCALE = 1.0 / np.sqrt(D)  # 1/32

_NC = None
LAST_RESULTS = None


def _build_nc():
    BF = mybir.dt.bfloat16
    R = mybir.dt.float32r
    F = mybir.dt.float32
    nc = bacc.Bacc("TRN2", target_bir_lowering=False, debug=False, num_devices=NCORES)

    KD = D // 128        # 8 contraction chunks over d
    NJ = N // 128        # 16 key blocks (j)
    CH = 512             # i-chunk width (one PSUM bank)
    NCH = N // CH        # 4 chunks
    NSUB = CH // 128     # 4 row-subblocks per chunk

    xT = nc.dram_tensor("xT", [D, N], BF, kind="ExternalInput").ap()
    xn = nc.dram_tensor("xn", [N, D], BF, kind="ExternalInput").ap()
    amat = nc.dram_tensor("amat", [D, D], BF, kind="ExternalInput").ap()
    ub = nc.dram_tensor("ub", [128, KD], F, kind="ExternalInput").ap()
    ones = nc.dram_tensor("ones", [128, 8], R, kind="ExternalInput").ap()
    out = nc.dram_tensor("out", [N, D], F, kind="ExternalOutput").ap()

    with tile.TileContext(nc) as tc:
        with tc.tile_pool(name="pers", bufs=1) as pers, \
             tc.tile_pool(name="ob", bufs=2) as obp, \
             tc.tile_pool(name="rd", bufs=2) as rdp:

            xt = [pers.tile([128, N], BF, tag=f"xt{k}", name=f"xt{k}")
                  for k in range(KD)]
            xv = [pers.tile([128, D], BF, tag=f"xv{j}", name=f"xv{j}")
                  for j in range(NJ)]
            tT = [pers.tile([128, N], BF, tag=f"tT{k}", name=f"tT{k}")
                  for k in range(KD)]
            # f32r so the denominator ones-matmul can consume it directly
            # (the BIR verifier requires fp32r matmul inputs to be written
            # as fp32r; DVE rounds on write).
            esum = pers.tile([128, N], R, tag="esum", name="esum")
            ubt = pers.tile([128, KD], F, tag="ub", name="ubt")
            onesT = pers.tile([128, 8], R, tag="ones", name="onesT")
            nc.sync.dma_start(out=onesT, in_=ones)

            # ---------------- stage A: t^T = A^T X^T (+u) ----------------
            with tc.tile_pool(name="aw", bufs=1) as awp, \
                 tc.tile_pool(name="psA", bufs=2, space="PSUM") as psAp:
                aw = [awp.tile([128, D], BF, tag=f"aw{k}", name=f"aw{k}")
                      for k in range(KD)]
                # DMA priority: the (c=0, m=0) k-loop touches a_k[:,0:128]
                # and xt_k[:,0:512] for every k — land those first, then the
                # remainders, then stage-C-only tensors.
                for k in range(KD):
                    nc.sync.dma_start(out=aw[k][:, 0:128],
                                      in_=amat[k * 128:(k + 1) * 128, 0:128])
                    nc.sync.dma_start(out=xt[k][:, 0:CH],
                                      in_=xT[k * 128:(k + 1) * 128, 0:CH])
                for k in range(KD):
                    nc.sync.dma_start(out=aw[k][:, 128:D],
                                      in_=amat[k * 128:(k + 1) * 128, 128:D])
                    nc.sync.dma_start(out=xt[k][:, CH:N],
                                      in_=xT[k * 128:(k + 1) * 128, CH:N])
                nc.sync.dma_start(out=ubt, in_=ub)
                for j in range(NJ):
                    nc.sync.dma_start(out=xv[j], in_=xn[j * 128:(j + 1) * 128, :])

                for c in range(NCH):
                    cols = slice(c * CH, (c + 1) * CH)
                    for m in range(KD):
                        pt = psAp.tile([128, CH], F, tag="a", name="psA")
                        for k in range(KD):
                            nc.tensor.matmul(
                                pt, aw[k][:, m * 128:(m + 1) * 128],
                                xt[k][:, cols],
                                start=(k == 0), stop=(k == KD - 1))
                        nc.scalar.activation(
                            tT[m][:, cols], pt,
                            mybir.ActivationFunctionType.Identity,
                            bias=ubt[:, m:m + 1], scale=1.0)

            with tc.tile_pool(name="e", bufs=1) as epool:
                ee = [epool.tile([128, N], BF, tag=f"e{j}", name=f"e{j}")
                      for j in range(NJ)]

                # ------------- stage B: S^T strips + exp + esum -------------
                with tc.tile_pool(name="psB", bufs=2, space="PSUM") as psBp:
                    for j in range(NJ):
                        jb = slice(j * 128, (j + 1) * 128)
                        ps = [psBp.tile([128, CH], F, tag=f"b{c}", name=f"psB{c}")
                              for c in range(NCH)]
                        for k in range(KD):
                            for c in range(NCH):
                                nc.tensor.matmul(
                                    ps[c], xt[k][:, jb],
                                    tT[k][:, c * CH:(c + 1) * CH],
                                    start=(k == 0), stop=(k == KD - 1))
                        for c in range(NCH):
                            csl = slice(c * CH, (c + 1) * CH)
                            nc.scalar.activation(
                                ee[j][:, csl], ps[c],
                                mybir.ActivationFunctionType.Exp, scale=SCALE)
                            if j == 0:
                                nc.vector.tensor_copy(esum[:, csl], ee[0][:, csl])
                            else:
                                nc.vector.tensor_add(esum[:, csl], esum[:, csl],
                                                     ee[j][:, csl])

                # ------------- stage C: out rows = attn @ X -------------
                with tc.tile_pool(name="psO", bufs=2, space="PSUM") as psOp, \
                     tc.tile_pool(name="psD", bufs=2, space="PSUM") as psDp:
                    for c in range(NCH):
                        for sub in range(NSUB):
                            i0 = c * CH + sub * 128
                            isl = slice(i0, i0 + 128)
                            p0 = psOp.tile([128, 512], F, tag="p0", name="psO0")
                            p1 = psOp.tile([128, 512], F, tag="p1", name="psO1")
                            for j in range(NJ):
                                lhs = ee[j][:, isl]
                                nc.tensor.matmul(p0, lhs, xv[j][:, 0:512],
                                                 start=(j == 0), stop=(j == NJ - 1))
                                nc.tensor.matmul(p1, lhs, xv[j][:, 512:1024],
                                                 start=(j == 0), stop=(j == NJ - 1))
                            pd = psDp.tile([128, 8], F, tag="pd", name="psD")
                            nc.tensor.matmul(pd, esum[:, isl], onesT,
                                             start=True, stop=True)
                            rden = rdp.tile([128, 1], F, tag="rden", name="rden")
                            nc.vector.reciprocal(rden, pd[:, 0:1])
                            ob = obp.tile([128, D], F, tag="ob", name="ob")
                            nc.vector.tensor_scalar_mul(ob[:, 0:512], p0, rden)
                            nc.vector.tensor_scalar_mul(ob[:, 512:1024], p1, rden)
                            nc.sync.dma_start(out=out[i0:i0 + 128, :], in_=ob)

    nc.finalize()
    return nc


def _get_nc():
    global _NC
    if _NC is None:
        _NC = _build_nc()
    return _NC


def _prep_shared(W_qk, b_qk):
    W_qk = np.ascontiguousarray(W_qk, dtype=np.float32)
    b_qk = np.asarray(b_qk, dtype=np.float32)
    Wq, Wk = W_qk[:D], W_qk[D:]
    bk = b_qk[D:]
    amat = np.ascontiguousarray((Wk.T @ Wq).astype(ml_dtypes.bfloat16))
    u = Wq.T @ bk  # [D]; the bq/i-dependent dot terms cancel in softmax
    ub = np.ascontiguousarray(u.reshape(KD_HOST, 128).T, dtype=np.float32)
    return amat, ub


KD_HOST = D // 128


def kernel(x: np.ndarray, W_qk: np.ndarray, b_qk: np.ndarray) -> np.ndarray:
    global LAST_RESULTS
    assert x.shape == (B, N, D), x.shape
    nc = _get_nc()

    x = np.ascontiguousarray(x, dtype=np.float32)
    amat, ub = _prep_shared(W_qk, b_qk)
    in_maps = []
    for c in range(NCORES):
        xb = x[c]
        in_maps.append({
            "xT": np.ascontiguousarray(xb.T.astype(ml_dtypes.bfloat16)),
            "xn": np.ascontiguousarray(xb.astype(ml_dtypes.bfloat16)),
            "amat": amat,
            "ub": ub,
            "ones": np.ones((128, 8), dtype=np.float32),
        })

    res = run_bass_kernel_spmd(nc, in_maps, core_ids=list(range(NCORES)))
    LAST_RESULTS = res
    out = np.stack([res.results[c]["out"] for c in range(NCORES)], axis=0)
    return out.astype(np.float32)


if __name__ == "__main__":
    rng = np.random.default_rng(0)
    x = rng.standard_normal((B, N, D), dtype=np.float32)
    limit = float(np.sqrt(6.0 / (D + 2 * D)))
    W = rng.uniform(-limit, limit, size=(2 * D, D)).astype(np.float32)
    b = np.zeros((2 * D,), dtype=np.float32)
    got = kernel(x, W, b)
    print("out", got.shape, got.dtype)


# revision 15
# speedup vs baseline: 1.2651x; 1.0632x over previous
"""Trainium2 Bass kernel for single-head attention (no V projection).

Reference computation (per batch b):
    qk   = x @ W_qk.T + b_qk          # [n, 2d]
    q, k = qk[:, :d], qk[:, d:]
    dots[i, j] = k_i . q_j / sqrt(d)
    attn = softmax(dots, axis=-1)
    out[i] = sum_j attn[i, j] * x[j]

Key algebraic folding: dots[i,j] = (Wk x_i + bk).(Wq x_j + bq).  The terms
that depend only on i are constant along the softmax axis j and cancel, so
    softmax_j(dots[i,:]) == softmax_j(x_i^T A x_j + u.x_j),
with A = Wk^T Wq [d,d] and u = Wq^T bk, both input-independent and folded on
the host.  This removes the separate q/k projections: the device computes
    t   = X A + u                     # [n, d]   (stage A; u via ACT bias)
    S^T = X t^T                       # [n, n]   (stage B)
    E   = exp(S^T / 32)               # ACT, no max-subtraction (scores are
                                      #   ~N(0,0.67); exp cannot overflow)
    out = normalize(E^T X)            # (stage C; denominator via a DVE
                                      #   running sum + tiny ones-matmul)
cutting device matmul work from 3 N^2 d-class GEMMs + projections (1552
512-row matmuls) to 1296.

All matmul operands are bfloat16 (same PE rate as float32r on TRN2, half the
SBUF/DMA) — measured end-to-end error ~3e-3 scaled vs the 2e-2 gate.  Both x
layouts (xT for stages A/B, xn for stage C) and A stay SBUF-resident; there
is no DRAM spill.  Every stationary [128,128] block is loaded exactly once
(LDWEIGHTS is elided across the 2-4 matmuls sharing it and hides under the
matmul stream).

Sharding: data-parallel over batch b (8 batches -> 8 NeuronCores), no
collectives.
"""
import sys

try:
    import concourse.bass as bass  # noqa: F401
except ImportError:  # pragma: no cover
    sys.path.insert(0, "/opt/trn_rl_repo")

import numpy as np
import ml_dtypes
import concourse.bass as bass
import concourse.mybir as mybir
import concourse.tile as tile
from concourse import bacc
from concourse.bass_utils import run_bass_kernel_spmd
import concourse.bass_utils as _bu

# NOTE: walrus --enable-ldw-opt=true (used by the f32r baseline) is
# incompatible with the explicit InstLdweights that bacc synthesizes for
# multi-wait bf16 matmuls; keep the default (false).

B, N, D = 8, 2048, 1024
NCORES = 8
SCALE = 1.0 / np.sqrt(D)  # 1/32

_NC = None
LAST_RESULTS = None


def _build_nc():
    BF = mybir.dt.bfloat16
    R = mybir.dt.float32r
    F = mybir.dt.float32
    nc = bacc.Bacc("TRN2", target_bir_lowering=False, debug=False, num_devices=NCORES)

    KD = D // 128        # 8 contraction chunks over d
    NJ = N // 128        # 16 key blocks (j)
    CH = 512             # i-chunk width (one PSUM bank)
    NCH = N // CH        # 4 chunks
    NSUB = CH // 128     # 4 row-subblocks per chunk

    # Host-prepared partition-major layouts (lines of 1-2KB for DMA):
    #   xt4[p, k, i] = x[i, k*128+p];  xn4[p, j, d] = x[j*128+p, d];
    #   aw4[p, k, e] = A[k*128+p, e]
    xt4 = nc.dram_tensor("xt4", [128, KD * N], BF, kind="ExternalInput").ap()
    xn4 = nc.dram_tensor("xn4", [128, NJ * D], BF, kind="ExternalInput").ap()
    aw4 = nc.dram_tensor("aw4", [128, KD * D], BF, kind="ExternalInput").ap()
    ub = nc.dram_tensor("ub", [128, KD], F, kind="ExternalInput").ap()
    ones = nc.dram_tensor("ones", [128, 8], R, kind="ExternalInput").ap()
    out = nc.dram_tensor("out", [N, D], F, kind="ExternalOutput").ap()
    xt4r = xt4.rearrange("p (k i) -> p k i", k=KD)
    xn4r = xn4.rearrange("p (j d) -> p j d", j=NJ)
    aw4r = aw4.rearrange("p (k e) -> p k e", k=KD)

    with tile.TileContext(nc) as tc:
        with tc.tile_pool(name="pers", bufs=1) as pers, \
             tc.tile_pool(name="ob", bufs=2) as obp, \
             tc.tile_pool(name="rd", bufs=2) as rdp:

            xt_all = pers.tile([128, KD * N], BF, tag="xt", name="xt_all")
            xtr = xt_all.rearrange("p (k i) -> p k i", k=KD)
            xt = [xtr[:, k] for k in range(KD)]
            xv_all = pers.tile([128, NJ * D], BF, tag="xv", name="xv_all")
            xvr = xv_all.rearrange("p (j d) -> p j d", j=NJ)
            xv = [xvr[:, j] for j in range(NJ)]
            tT = [pers.tile([128, N], BF, tag=f"tT{k}", name=f"tT{k}")
                  for k in range(KD)]
            # f32r so the denominator ones-matmul can consume it directly
            # (the BIR verifier requires fp32r matmul inputs to be written
            # as fp32r; DVE rounds on write).
            esum = pers.tile([128, N], R, tag="esum", name="esum")
            ubt = pers.tile([128, KD], F, tag="ub", name="ubt")
            onesT = pers.tile([128, 8], R, tag="ones", name="onesT")

            # ---------------- stage A: t^T = A^T X^T (+u) ----------------
            with tc.tile_pool(name="aw", bufs=1) as awp, \
                 tc.tile_pool(name="psA", bufs=2, space="PSUM") as psAp:
                aw_all = awp.tile([128, KD * D], BF, tag="aw", name="aw_all")
                awr = aw_all.rearrange("p (k e) -> p k e", k=KD)
                # DMA priority order (one in-order HW queue, ~0.65us per
                # descriptor issue): tiny bias/ones first (the very first
                # ACT drain needs ubt), then exactly what the (c0, m)
                # blocks consume in order, then stage-C-only tensors.
                nc.sync.dma_start(out=ubt, in_=ub)
                nc.sync.dma_start(out=onesT, in_=ones)
                nc.sync.dma_start(out=awr[:, :, 0:128], in_=aw4r[:, :, 0:128])
                nc.sync.dma_start(out=xtr[:, :, 0:CH], in_=xt4r[:, :, 0:CH])
                for m in range(1, KD):
                    nc.sync.dma_start(out=awr[:, :, m * 128:(m + 1) * 128],
                                      in_=aw4r[:, :, m * 128:(m + 1) * 128])
                for c in range(1, NCH):
                    nc.sync.dma_start(out=xtr[:, :, c * CH:(c + 1) * CH],
                                      in_=xt4r[:, :, c * CH:(c + 1) * CH])
                nc.sync.dma_start(out=xvr[:, 0:NJ // 2], in_=xn4r[:, 0:NJ // 2])
                nc.sync.dma_start(out=xvr[:, NJ // 2:NJ], in_=xn4r[:, NJ // 2:NJ])

                for c in range(NCH):
                    cols = slice(c * CH, (c + 1) * CH)
                    for m in range(KD):
                        pt = psAp.tile([128, CH], F, tag="a", name="psA")
                        for k in range(KD):
                            nc.tensor.matmul(
                                pt, awr[:, k, m * 128:(m + 1) * 128],
                                xt[k][:, cols],
                                start=(k == 0), stop=(k == KD - 1))
                        nc.scalar.activation(
                            tT[m][:, cols], pt,
                            mybir.ActivationFunctionType.Identity,
                            bias=ubt[:, m:m + 1], scale=1.0)

            with tc.tile_pool(name="e", bufs=1) as epool:
                ee = [epool.tile([128, N], BF, tag=f"e{j}", name=f"e{j}")
                      for j in range(NJ)]

                # ------------- stage B: S^T strips + exp + esum -------------
                with tc.tile_pool(name="psB", bufs=2, space="PSUM") as psBp:
                    for j in range(NJ):
                        jb = slice(j * 128, (j + 1) * 128)
                        ps = [psBp.tile([128, CH], F, tag=f"b{c}", name=f"psB{c}")
                              for c in range(NCH)]
                        for k in range(KD):
                            for c in range(NCH):
                                nc.tensor.matmul(
                                    ps[c], xt[k][:, jb],
                                    tT[k][:, c * CH:(c + 1) * CH],
                                    start=(k == 0), stop=(k == KD - 1))
                        for c in range(NCH):
                            csl = slice(c * CH, (c + 1) * CH)
                            nc.scalar.activation(
                                ee[j][:, csl], ps[c],
                                mybir.ActivationFunctionType.Exp, scale=SCALE)
                            if j == 0:
                                nc.vector.tensor_copy(esum[:, csl], ee[0][:, csl])
                            else:
                                nc.vector.tensor_add(esum[:, csl], esum[:, csl],
                                                     ee[j][:, csl])

                # ------------- stage C: out rows = attn @ X -------------
                with tc.tile_pool(name="psO", bufs=2, space="PSUM") as psOp, \
                     tc.tile_pool(name="psD", bufs=2, space="PSUM") as psDp:
                    for c in range(NCH):
                        for sub in range(NSUB):
                            i0 = c * CH + sub * 128
                            isl = slice(i0, i0 + 128)
                            p0 = psOp.tile([128, 512], F, tag="p0", name="psO0")
                            p1 = psOp.tile([128, 512], F, tag="p1", name="psO1")
                            for j in range(NJ):
                                lhs = ee[j][:, isl]
                                nc.tensor.matmul(p0, lhs, xv[j][:, 0:512],
                                                 start=(j == 0), stop=(j == NJ - 1))
                                nc.tensor.matmul(p1, lhs, xv[j][:, 512:1024],
                                                 start=(j == 0), stop=(j == NJ - 1))
                            pd = psDp.tile([128, 8], F, tag="pd", name="psD")
                            nc.tensor.matmul(pd, esum[:, isl], onesT,
                                             start=True, stop=True)
                            rden = rdp.tile([128, 1], F, tag="rden", name="rden")
                            nc.vector.reciprocal(rden, pd[:, 0:1])
                            # split halves so the last transfer overlaps the
                            # second normalize (input DMAs are long done, so
                            # the sync queue is idle here).
                            ob = obp.tile([128, D], F, tag="ob", name="ob")
                            nc.vector.tensor_scalar_mul(ob[:, 0:512], p0, rden)
                            nc.sync.dma_start(out=out[i0:i0 + 128, 0:512],
                                              in_=ob[:, 0:512])
                            nc.vector.tensor_scalar_mul(ob[:, 512:1024], p1, rden)
                            nc.sync.dma_start(out=out[i0:i0 + 128, 512:1024],
                                              in_=ob[:, 512:1024])

    nc.finalize()
    return nc


def _get_nc():
    global _NC
    if _NC is None:
        _NC = _build_nc()
    return _NC


KD_HOST = D // 128
NJ_HOST = N // 128


def _prep_shared(W_qk, b_qk):
    W_qk = np.ascontiguousarray(W_qk, dtype=np.float32)
    b_qk = np.asarray(b_qk, dtype=np.float32)
    Wq, Wk = W_qk[:D], W_qk[D:]
    bk = b_qk[D:]
    A = (Wk.T @ Wq).astype(ml_dtypes.bfloat16)
    # aw4[p, k, e] = A[k*128+p, e]
    aw4 = np.ascontiguousarray(
        A.reshape(KD_HOST, 128, D).transpose(1, 0, 2).reshape(128, -1))
    u = Wq.T @ bk  # [D]; the bq/i-dependent dot terms cancel in softmax
    ub = np.ascontiguousarray(u.reshape(KD_HOST, 128).T, dtype=np.float32)
    return aw4, ub


def kernel(x: np.ndarray, W_qk: np.ndarray, b_qk: np.ndarray) -> np.ndarray:
    global LAST_RESULTS
    assert x.shape == (B, N, D), x.shape
    nc = _get_nc()

    x = np.ascontiguousarray(x, dtype=np.float32)
    aw4, ub = _prep_shared(W_qk, b_qk)
    ones = np.ones((128, 8), dtype=np.float32)
    in_maps = []
    for c in range(NCORES):
        xb = x[c].astype(ml_dtypes.bfloat16)
        # xt4[p, k, i] = x[i, k*128+p]; xn4[p, j, d] = x[j*128+p, d]
        xt4 = np.ascontiguousarray(
            xb.T.reshape(KD_HOST, 128, N).transpose(1, 0, 2).reshape(128, -1))
        xn4 = np.ascontiguousarray(
            xb.reshape(NJ_HOST, 128, D).transpose(1, 0, 2).reshape(128, -1))
        in_maps.append({
            "xt4": xt4,
            "xn4": xn4,
            "aw4": aw4,
            "ub": ub,
            "ones": ones,
        })

    res = run_bass_kernel_spmd(nc, in_maps, core_ids=list(range(NCORES)))
    LAST_RESULTS = res
    out = np.stack([res.results[c]["out"] for c in range(NCORES)], axis=0)
    return out.astype(np.float32)


if __name__ == "__main__":
    rng = np.random.default_rng(0)
    x = rng.standard_normal((B, N, D), dtype=np.float32)
    limit = float(np.sqrt(6.0 / (D + 2 * D)))
    W = rng.uniform(-limit, limit, size=(2 * D, D)).astype(np.float32)
    b = np.zeros((2 * D,), dtype=np.float32)
    got = kernel(x, W, b)
    print("out", got.shape, got.dtype)


# revision 17
# speedup vs baseline: 1.2771x; 1.0095x over previous
"""Trainium2 Bass kernel for single-head attention (no V projection).

Reference computation (per batch b):
    qk   = x @ W_qk.T + b_qk          # [n, 2d]
    q, k = qk[:, :d], qk[:, d:]
    dots[i, j] = k_i . q_j / sqrt(d)
    attn = softmax(dots, axis=-1)
    out[i] = sum_j attn[i, j] * x[j]

Key algebraic folding: dots[i,j] = (Wk x_i + bk).(Wq x_j + bq).  The terms
that depend only on i are constant along the softmax axis j and cancel, so
    softmax_j(dots[i,:]) == softmax_j(x_i^T A x_j + u.x_j),
with A = Wk^T Wq [d,d] and u = Wq^T bk, both input-independent and folded on
the host.  This removes the separate q/k projections: the device computes
    t   = X A + u                     # [n, d]   (stage A; u via ACT bias)
    S^T = X t^T                       # [n, n]   (stage B)
    E   = exp(S^T / 32)               # ACT, no max-subtraction (scores are
                                      #   ~N(0,0.67); exp cannot overflow)
    out = normalize(E^T X)            # (stage C; denominator via a DVE
                                      #   running sum + tiny ones-matmul)
cutting device matmul work from 3 N^2 d-class GEMMs + projections (1552
512-row matmuls) to 1296.

All matmul operands are bfloat16 (same PE rate as float32r on TRN2, half the
SBUF/DMA) — measured end-to-end error ~3e-3 scaled vs the 2e-2 gate.  Both x
layouts (xT for stages A/B, xn for stage C) and A stay SBUF-resident; there
is no DRAM spill.  Every stationary [128,128] block is loaded exactly once
(LDWEIGHTS is elided across the 2-4 matmuls sharing it and hides under the
matmul stream).

Sharding: data-parallel over batch b (8 batches -> 8 NeuronCores), no
collectives.
"""
import sys

try:
    import concourse.bass as bass  # noqa: F401
except ImportError:  # pragma: no cover
    sys.path.insert(0, "/opt/trn_rl_repo")

import numpy as np
import ml_dtypes
import concourse.bass as bass
import concourse.mybir as mybir
import concourse.tile as tile
from concourse import bacc
from concourse.bass_utils import run_bass_kernel_spmd
import concourse.bass_utils as _bu

# NOTE: walrus --enable-ldw-opt=true (used by the f32r baseline) is
# incompatible with the explicit InstLdweights that bacc synthesizes for
# multi-wait bf16 matmuls; keep the default (false).

B, N, D = 8, 2048, 1024
NCORES = 8
SCALE = 1.0 / np.sqrt(D)  # 1/32

_NC = None
LAST_RESULTS = None


def _build_nc():
    BF = mybir.dt.bfloat16
    R = mybir.dt.float32r
    F = mybir.dt.float32
    nc = bacc.Bacc("TRN2", target_bir_lowering=False, debug=False, num_devices=NCORES)

    KD = D // 128        # 8 contraction chunks over d
    NJ = N // 128        # 16 key blocks (j)
    CH = 512             # i-chunk width (one PSUM bank)
    NCH = N // CH        # 4 chunks
    NSUB = CH // 128     # 4 row-subblocks per chunk

    # Host-prepared partition-major layouts (lines of 1-2KB for DMA):
    #   xt4[p, k, i] = x[i, k*128+p];  xn4[p, j, d] = x[j*128+p, d];
    #   aw4[p, k, e] = A[k*128+p, e]
    xt4 = nc.dram_tensor("xt4", [128, KD * N], BF, kind="ExternalInput").ap()
    xn4 = nc.dram_tensor("xn4", [128, NJ * D], BF, kind="ExternalInput").ap()
    aw4 = nc.dram_tensor("aw4", [128, KD * D], BF, kind="ExternalInput").ap()
    ub = nc.dram_tensor("ub", [128, KD], F, kind="ExternalInput").ap()
    ones = nc.dram_tensor("ones", [128, 8], R, kind="ExternalInput").ap()
    out = nc.dram_tensor("out", [N, D], F, kind="ExternalOutput").ap()
    xt4r = xt4.rearrange("p (k i) -> p k i", k=KD)
    xn4r = xn4.rearrange("p (j d) -> p j d", j=NJ)
    aw4r = aw4.rearrange("p (k e) -> p k e", k=KD)

    with tile.TileContext(nc) as tc:
        with tc.tile_pool(name="pers", bufs=1) as pers, \
             tc.tile_pool(name="ob", bufs=2) as obp, \
             tc.tile_pool(name="rd", bufs=2) as rdp:

            xt_all = pers.tile([128, KD * N], BF, tag="xt", name="xt_all")
            xtr = xt_all.rearrange("p (k i) -> p k i", k=KD)
            xt = [xtr[:, k] for k in range(KD)]
            xv_all = pers.tile([128, NJ * D], BF, tag="xv", name="xv_all")
            xvr = xv_all.rearrange("p (j d) -> p j d", j=NJ)
            xv = [xvr[:, j] for j in range(NJ)]
            tT = [pers.tile([128, N], BF, tag=f"tT{k}", name=f"tT{k}")
                  for k in range(KD)]
            # f32r so the denominator ones-matmul can consume it directly
            # (the BIR verifier requires fp32r matmul inputs to be written
            # as fp32r; DVE rounds on write).
            esum = pers.tile([128, N], R, tag="esum", name="esum")
            ubt = pers.tile([128, KD], F, tag="ub", name="ubt")
            onesT = pers.tile([128, 8], R, tag="ones", name="onesT")
            warm = pers.tile([128, CH], BF, tag="warm", name="warm")
            nc.vector.memset(warm, 1.0)

            # ---------------- stage A: t^T = A^T X^T (+u) ----------------
            with tc.tile_pool(name="aw", bufs=1) as awp, \
                 tc.tile_pool(name="psA", bufs=2, space="PSUM") as psAp:
                aw_all = awp.tile([128, KD * D], BF, tag="aw", name="aw_all")
                awr = aw_all.rearrange("p (k e) -> p k e", k=KD)

                # PE p-state warm-up: ~10 dependency-free matmuls on garbage
                # SBUF fill the otherwise idle DMA head window (~7-10us) and
                # bring the PE to full clock before the first real matmul.
                for w in range(10):
                    wp = psAp.tile([128, CH], F, tag="warm", name="psW")
                    nc.tensor.matmul(wp, warm[:, 0:128], warm,
                                     start=True, stop=True)

                # DMA priority order (one in-order HW queue, ~0.7us per
                # descriptor issue): exactly what the (c0, m) blocks consume,
                # in consumption order; bias right after the m0 stream (first
                # ACT drain needs it); stage-C tensors last.
                nc.sync.dma_start(out=awr[:, :, 0:128], in_=aw4r[:, :, 0:128])
                nc.sync.dma_start(out=xtr[:, 0:1, 0:CH], in_=xt4r[:, 0:1, 0:CH])
                nc.sync.dma_start(out=xtr[:, 1:KD, 0:CH], in_=xt4r[:, 1:KD, 0:CH])
                nc.sync.dma_start(out=ubt, in_=ub)
                nc.sync.dma_start(out=onesT, in_=ones)
                for m in range(1, KD):
                    nc.sync.dma_start(out=awr[:, :, m * 128:(m + 1) * 128],
                                      in_=aw4r[:, :, m * 128:(m + 1) * 128])
                for c in range(1, NCH):
                    nc.sync.dma_start(out=xtr[:, :, c * CH:(c + 1) * CH],
                                      in_=xt4r[:, :, c * CH:(c + 1) * CH])
                nc.sync.dma_start(out=xvr[:, 0:NJ // 2], in_=xn4r[:, 0:NJ // 2])
                nc.sync.dma_start(out=xvr[:, NJ // 2:NJ], in_=xn4r[:, NJ // 2:NJ])

                for c in range(NCH):
                    cols = slice(c * CH, (c + 1) * CH)
                    for m in range(KD):
                        pt = psAp.tile([128, CH], F, tag="a", name="psA")
                        for k in range(KD):
                            nc.tensor.matmul(
                                pt, awr[:, k, m * 128:(m + 1) * 128],
                                xt[k][:, cols],
                                start=(k == 0), stop=(k == KD - 1))
                        nc.scalar.activation(
                            tT[m][:, cols], pt,
                            mybir.ActivationFunctionType.Identity,
                            bias=ubt[:, m:m + 1], scale=1.0)

            with tc.tile_pool(name="e", bufs=1) as epool:
                ee = [epool.tile([128, N], BF, tag=f"e{j}", name=f"e{j}")
                      for j in range(NJ)]

                # ------------- stage B: S^T strips + exp + esum -------------
                with tc.tile_pool(name="psB", bufs=2, space="PSUM") as psBp:
                    for j in range(NJ):
                        jb = slice(j * 128, (j + 1) * 128)
                        ps = [psBp.tile([128, CH], F, tag=f"b{c}", name=f"psB{c}")
                              for c in range(NCH)]
                        for k in range(KD):
                            for c in range(NCH):
                                nc.tensor.matmul(
                                    ps[c], xt[k][:, jb],
                                    tT[k][:, c * CH:(c + 1) * CH],
                                    start=(k == 0), stop=(k == KD - 1))
                        for c in range(NCH):
                            csl = slice(c * CH, (c + 1) * CH)
                            nc.scalar.activation(
                                ee[j][:, csl], ps[c],
                                mybir.ActivationFunctionType.Exp, scale=SCALE)
                            if j == 0:
                                nc.vector.tensor_copy(esum[:, csl], ee[0][:, csl])
                            else:
                                nc.vector.tensor_add(esum[:, csl], esum[:, csl],
                                                     ee[j][:, csl])

                # ------------- stage C: out rows = attn @ X -------------
                with tc.tile_pool(name="psO", bufs=2, space="PSUM") as psOp, \
                     tc.tile_pool(name="psD", bufs=2, space="PSUM") as psDp:
                    for c in range(NCH):
                        for sub in range(NSUB):
                            i0 = c * CH + sub * 128
                            isl = slice(i0, i0 + 128)
                            p0 = psOp.tile([128, 512], F, tag="p0", name="psO0")
                            p1 = psOp.tile([128, 512], F, tag="p1", name="psO1")
                            for j in range(NJ):
                                lhs = ee[j][:, isl]
                                nc.tensor.matmul(p0, lhs, xv[j][:, 0:512],
                                                 start=(j == 0), stop=(j == NJ - 1))
                                nc.tensor.matmul(p1, lhs, xv[j][:, 512:1024],
                                                 start=(j == 0), stop=(j == NJ - 1))
                            pd = psDp.tile([128, 8], F, tag="pd", name="psD")
                            nc.tensor.matmul(pd, esum[:, isl], onesT,
                                             start=True, stop=True)
                            rden = rdp.tile([128, 1], F, tag="rden", name="rden")
                            nc.vector.reciprocal(rden, pd[:, 0:1])
                            # split halves so the last transfer overlaps the
                            # second normalize (input DMAs are long done, so
                            # the sync queue is idle here).
                            ob = obp.tile([128, D], F, tag="ob", name="ob")
                            nc.vector.tensor_scalar_mul(ob[:, 0:512], p0, rden)
                            nc.sync.dma_start(out=out[i0:i0 + 128, 0:512],
                                              in_=ob[:, 0:512])
                            nc.vector.tensor_scalar_mul(ob[:, 512:1024], p1, rden)
                            nc.sync.dma_start(out=out[i0:i0 + 128, 512:1024],
                                              in_=ob[:, 512:1024])

    nc.finalize()
    return nc


def _get_nc():
    global _NC
    if _NC is None:
        _NC = _build_nc()
    return _NC


KD_HOST = D // 128
NJ_HOST = N // 128


def _prep_shared(W_qk, b_qk):
    W_qk = np.ascontiguousarray(W_qk, dtype=np.float32)
    b_qk = np.asarray(b_qk, dtype=np.float32)
    Wq, Wk = W_qk[:D], W_qk[D:]
    bk = b_qk[D:]
    A = (Wk.T @ Wq).astype(ml_dtypes.bfloat16)
    # aw4[p, k, e] = A[k*128+p, e]
    aw4 = np.ascontiguousarray(
        A.reshape(KD_HOST, 128, D).transpose(1, 0, 2).reshape(128, -1))
    u = Wq.T @ bk  # [D]; the bq/i-dependent dot terms cancel in softmax
    ub = np.ascontiguousarray(u.reshape(KD_HOST, 128).T, dtype=np.float32)
    return aw4, ub


def kernel(x: np.ndarray, W_qk: np.ndarray, b_qk: np.ndarray) -> np.ndarray:
    global LAST_RESULTS
    assert x.shape == (B, N, D), x.shape
    nc = _get_nc()

    x = np.ascontiguousarray(x, dtype=np.float32)
    aw4, ub = _prep_shared(W_qk, b_qk)
    ones = np.ones((128, 8), dtype=np.float32)
    in_maps = []
    for c in range(NCORES):
        xb = x[c].astype(ml_dtypes.bfloat16)
        # xt4[p, k, i] = x[i, k*128+p]; xn4[p, j, d] = x[j*128+p, d]
        xt4 = np.ascontiguousarray(
            xb.T.reshape(KD_HOST, 128, N).transpose(1, 0, 2).reshape(128, -1))
        xn4 = np.ascontiguousarray(
            xb.reshape(NJ_HOST, 128, D).transpose(1, 0, 2).reshape(128, -1))
        in_maps.append({
            "xt4": xt4,
            "xn4": xn4,
            "aw4": aw4,
            "ub": ub,
            "ones": ones,
        })

    res = run_bass_kernel_spmd(nc, in_maps, core_ids=list(range(NCORES)))
    LAST_RESULTS = res
    out = np.stack([res.results[c]["out"] for c in range(NCORES)], axis=0)
    return out.astype(np.float32)


if __name__ == "__main__":
    rng = np.random.default_rng(0)
    x = rng.standard_normal((B, N, D), dtype=np.float32)
    limit = float(np.sqrt(6.0 / (D + 2 * D)))
    W = rng.uniform(-limit, limit, size=(2 * D, D)).astype(np.float32)
    b = np.zeros((2 * D,), dtype=np.float32)
    got = kernel(x, W, b)
    print("out", got.shape, got.dtype)


# revision 18
# speedup vs baseline: 1.2818x; 1.0036x over previous
"""Trainium2 Bass kernel for single-head attention (no V projection).

Reference computation (per batch b):
    qk   = x @ W_qk.T + b_qk          # [n, 2d]
    q, k = qk[:, :d], qk[:, d:]
    dots[i, j] = k_i . q_j / sqrt(d)
    attn = softmax(dots, axis=-1)
    out[i] = sum_j attn[i, j] * x[j]

Key algebraic folding: dots[i,j] = (Wk x_i + bk).(Wq x_j + bq).  The terms
that depend only on i are constant along the softmax axis j and cancel, so
    softmax_j(dots[i,:]) == softmax_j(x_i^T A x_j + u.x_j),
with A = Wk^T Wq [d,d] and u = Wq^T bk, both input-independent and folded on
the host.  This removes the separate q/k projections: the device computes
    t   = X A + u                     # [n, d]   (stage A; u via ACT bias)
    S^T = X t^T                       # [n, n]   (stage B)
    E   = exp(S^T / 32)               # ACT, no max-subtraction (scores are
                                      #   ~N(0,0.67); exp cannot overflow)
    out = normalize(E^T X)            # (stage C; denominator via a DVE
                                      #   running sum + tiny ones-matmul)
cutting device matmul work from 3 N^2 d-class GEMMs + projections (1552
512-row matmuls) to 1296.

All matmul operands are bfloat16 (same PE rate as float32r on TRN2, half the
SBUF/DMA) — measured end-to-end error ~3e-3 scaled vs the 2e-2 gate.  Both x
layouts (xT for stages A/B, xn for stage C) and A stay SBUF-resident; there
is no DRAM spill.  Every stationary [128,128] block is loaded exactly once
(LDWEIGHTS is elided across the 2-4 matmuls sharing it and hides under the
matmul stream).

Sharding: data-parallel over batch b (8 batches -> 8 NeuronCores), no
collectives.
"""
import sys

try:
    import concourse.bass as bass  # noqa: F401
except ImportError:  # pragma: no cover
    sys.path.insert(0, "/opt/trn_rl_repo")

import numpy as np
import ml_dtypes
import concourse.bass as bass
import concourse.mybir as mybir
import concourse.tile as tile
from concourse import bacc
from concourse.bass_utils import run_bass_kernel_spmd
import concourse.bass_utils as _bu

# NOTE: walrus --enable-ldw-opt=true (used by the f32r baseline) is
# incompatible with the explicit InstLdweights that bacc synthesizes for
# multi-wait bf16 matmuls; keep the default (false).

B, N, D = 8, 2048, 1024
NCORES = 8
SCALE = 1.0 / np.sqrt(D)  # 1/32

_NC = None
LAST_RESULTS = None


def _build_nc():
    BF = mybir.dt.bfloat16
    R = mybir.dt.float32r
    F = mybir.dt.float32
    nc = bacc.Bacc("TRN2", target_bir_lowering=False, debug=False, num_devices=NCORES)

    KD = D // 128        # 8 contraction chunks over d
    NJ = N // 128        # 16 key blocks (j)
    CH = 512             # i-chunk width (one PSUM bank)
    NCH = N // CH        # 4 chunks
    NSUB = CH // 128     # 4 row-subblocks per chunk

    # Host-prepared partition-major layouts (lines of 1-2KB for DMA):
    #   xt4[p, k, i] = x[i, k*128+p];  xn4[p, j, d] = x[j*128+p, d];
    #   aw4[p, k, e] = A[k*128+p, e]
    xt4 = nc.dram_tensor("xt4", [128, KD * N], BF, kind="ExternalInput").ap()
    xn4 = nc.dram_tensor("xn4", [128, NJ * D], BF, kind="ExternalInput").ap()
    aw4 = nc.dram_tensor("aw4", [128, KD * D], BF, kind="ExternalInput").ap()
    ub = nc.dram_tensor("ub", [128, KD], F, kind="ExternalInput").ap()
    ones = nc.dram_tensor("ones", [128, 8], R, kind="ExternalInput").ap()
    out = nc.dram_tensor("out", [N, D], F, kind="ExternalOutput").ap()
    xt4r = xt4.rearrange("p (k i) -> p k i", k=KD)
    xn4r = xn4.rearrange("p (j d) -> p j d", j=NJ)
    aw4r = aw4.rearrange("p (k e) -> p k e", k=KD)

    with tile.TileContext(nc) as tc:
        with tc.tile_pool(name="pers", bufs=1) as pers, \
             tc.tile_pool(name="ob", bufs=2) as obp, \
             tc.tile_pool(name="rd", bufs=2) as rdp:

            xt_all = pers.tile([128, KD * N], BF, tag="xt", name="xt_all")
            xtr = xt_all.rearrange("p (k i) -> p k i", k=KD)
            xt = [xtr[:, k] for k in range(KD)]
            xv_all = pers.tile([128, NJ * D], BF, tag="xv", name="xv_all")
            xvr = xv_all.rearrange("p (j d) -> p j d", j=NJ)
            xv = [xvr[:, j] for j in range(NJ)]
            tT = [pers.tile([128, N], BF, tag=f"tT{k}", name=f"tT{k}")
                  for k in range(KD)]
            # f32r so the denominator ones-matmul can consume it directly
            # (the BIR verifier requires fp32r matmul inputs to be written
            # as fp32r; DVE rounds on write).
            esum = pers.tile([128, N], R, tag="esum", name="esum")
            ubt = pers.tile([128, KD], F, tag="ub", name="ubt")
            onesT = pers.tile([128, 8], R, tag="ones", name="onesT")
            warm = pers.tile([128, CH], BF, tag="warm", name="warm")
            nc.vector.memset(warm, 1.0)

            # ---------------- stage A: t^T = A^T X^T (+u) ----------------
            with tc.tile_pool(name="aw", bufs=1) as awp, \
                 tc.tile_pool(name="psA", bufs=2, space="PSUM") as psAp:
                aw_all = awp.tile([128, KD * D], BF, tag="aw", name="aw_all")
                awr = aw_all.rearrange("p (k e) -> p k e", k=KD)

                # PE p-state warm-up: dependency-free matmuls on garbage SBUF
                # fill the otherwise idle DMA head window (~7.5-14.5us: the
                # first A block's aw-m0 + xt-c0 take that long to land) and
                # keep the PE at full clock so the first real matmuls run at
                # 216ns, not at the ~630ns re-ramp rate after an idle gap.
                for w in range(20):
                    wp = psAp.tile([128, CH], F, tag="warm", name="psW")
                    nc.tensor.matmul(wp, warm[:, 0:128], warm,
                                     start=True, stop=True)

                # DMA priority order (one in-order HW queue, ~0.7us per
                # descriptor issue): exactly what the (c0, m) blocks consume,
                # in consumption order; bias early (first ACT drain needs it)
                # but after the first-matmul critical pair; stage-C tensors
                # last.
                nc.sync.dma_start(out=awr[:, :, 0:128], in_=aw4r[:, :, 0:128])
                nc.sync.dma_start(out=xtr[:, :, 0:CH], in_=xt4r[:, :, 0:CH])
                nc.sync.dma_start(out=awr[:, :, 128:256], in_=aw4r[:, :, 128:256])
                nc.sync.dma_start(out=ubt, in_=ub)
                nc.sync.dma_start(out=onesT, in_=ones)
                for m in range(2, KD):
                    nc.sync.dma_start(out=awr[:, :, m * 128:(m + 1) * 128],
                                      in_=aw4r[:, :, m * 128:(m + 1) * 128])
                for c in range(1, NCH):
                    nc.sync.dma_start(out=xtr[:, :, c * CH:(c + 1) * CH],
                                      in_=xt4r[:, :, c * CH:(c + 1) * CH])
                nc.sync.dma_start(out=xvr[:, 0:NJ // 2], in_=xn4r[:, 0:NJ // 2])
                nc.sync.dma_start(out=xvr[:, NJ // 2:NJ], in_=xn4r[:, NJ // 2:NJ])

                for c in range(NCH):
                    cols = slice(c * CH, (c + 1) * CH)
                    for m in range(KD):
                        pt = psAp.tile([128, CH], F, tag="a", name="psA")
                        for k in range(KD):
                            nc.tensor.matmul(
                                pt, awr[:, k, m * 128:(m + 1) * 128],
                                xt[k][:, cols],
                                start=(k == 0), stop=(k == KD - 1))
                        nc.scalar.activation(
                            tT[m][:, cols], pt,
                            mybir.ActivationFunctionType.Identity,
                            bias=ubt[:, m:m + 1], scale=1.0)

            with tc.tile_pool(name="e", bufs=1) as epool:
                ee = [epool.tile([128, N], BF, tag=f"e{j}", name=f"e{j}")
                      for j in range(NJ)]

                # ------------- stage B: S^T strips + exp + esum -------------
                with tc.tile_pool(name="psB", bufs=2, space="PSUM") as psBp:
                    for j in range(NJ):
                        jb = slice(j * 128, (j + 1) * 128)
                        ps = [psBp.tile([128, CH], F, tag=f"b{c}", name=f"psB{c}")
                              for c in range(NCH)]
                        for k in range(KD):
                            for c in range(NCH):
                                nc.tensor.matmul(
                                    ps[c], xt[k][:, jb],
                                    tT[k][:, c * CH:(c + 1) * CH],
                                    start=(k == 0), stop=(k == KD - 1))
                        for c in range(NCH):
                            csl = slice(c * CH, (c + 1) * CH)
                            nc.scalar.activation(
                                ee[j][:, csl], ps[c],
                                mybir.ActivationFunctionType.Exp, scale=SCALE)
                            if j == 0:
                                nc.vector.tensor_copy(esum[:, csl], ee[0][:, csl])
                            else:
                                nc.vector.tensor_add(esum[:, csl], esum[:, csl],
                                                     ee[j][:, csl])

                # ------------- stage C: out rows = attn @ X -------------
                with tc.tile_pool(name="psO", bufs=2, space="PSUM") as psOp, \
                     tc.tile_pool(name="psD", bufs=2, space="PSUM") as psDp:
                    for c in range(NCH):
                        for sub in range(NSUB):
                            i0 = c * CH + sub * 128
                            isl = slice(i0, i0 + 128)
                            p0 = psOp.tile([128, 512], F, tag="p0", name="psO0")
                            p1 = psOp.tile([128, 512], F, tag="p1", name="psO1")
                            for j in range(NJ):
                                lhs = ee[j][:, isl]
                                nc.tensor.matmul(p0, lhs, xv[j][:, 0:512],
                                                 start=(j == 0), stop=(j == NJ - 1))
                                nc.tensor.matmul(p1, lhs, xv[j][:, 512:1024],
                                                 start=(j == 0), stop=(j == NJ - 1))
                            pd = psDp.tile([128, 8], F, tag="pd", name="psD")
                            nc.tensor.matmul(pd, esum[:, isl], onesT,
                                             start=True, stop=True)
                            rden = rdp.tile([128, 1], F, tag="rden", name="rden")
                            nc.vector.reciprocal(rden, pd[:, 0:1])
                            # split halves so the last transfer overlaps the
                            # second normalize (input DMAs are long done, so
                            # the sync queue is idle here).
                            ob = obp.tile([128, D], F, tag="ob", name="ob")
                            nc.vector.tensor_scalar_mul(ob[:, 0:512], p0, rden)
                            nc.sync.dma_start(out=out[i0:i0 + 128, 0:512],
                                              in_=ob[:, 0:512])
                            nc.vector.tensor_scalar_mul(ob[:, 512:1024], p1, rden)
                            nc.sync.dma_start(out=out[i0:i0 + 128, 512:1024],
                                              in_=ob[:, 512:1024])

    nc.finalize()
    return nc


def _get_nc():
    global _NC
    if _NC is None:
        _NC = _build_nc()
    return _NC


KD_HOST = D // 128
NJ_HOST = N // 128


def _prep_shared(W_qk, b_qk):
    W_qk = np.ascontiguousarray(W_qk, dtype=np.float32)
    b_qk = np.asarray(b_qk, dtype=np.float32)
    Wq, Wk = W_qk[:D], W_qk[D:]
    bk = b_qk[D:]
    A = (Wk.T @ Wq).astype(ml_dtypes.bfloat16)
    # aw4[p, k, e] = A[k*128+p, e]
    aw4 = np.ascontiguousarray(
        A.reshape(KD_HOST, 128, D).transpose(1, 0, 2).reshape(128, -1))
    u = Wq.T @ bk  # [D]; the bq/i-dependent dot terms cancel in softmax
    ub = np.ascontiguousarray(u.reshape(KD_HOST, 128).T, dtype=np.float32)
    return aw4, ub


def kernel(x: np.ndarray, W_qk: np.ndarray, b_qk: np.ndarray) -> np.ndarray:
    global LAST_RESULTS
    assert x.shape == (B, N, D), x.shape
    nc = _get_nc()

    x = np.ascontiguousarray(x, dtype=np.float32)
    aw4, ub = _prep_shared(W_qk, b_qk)
    ones = np.ones((128, 8), dtype=np.float32)
    in_maps = []
    for c in range(NCORES):
        xb = x[c].astype(ml_dtypes.bfloat16)
        # xt4[p, k, i] = x[i, k*128+p]; xn4[p, j, d] = x[j*128+p, d]
        xt4 = np.ascontiguousarray(
            xb.T.reshape(KD_HOST, 128, N).transpose(1, 0, 2).reshape(128, -1))
        xn4 = np.ascontiguousarray(
            xb.reshape(NJ_HOST, 128, D).transpose(1, 0, 2).reshape(128, -1))
        in_maps.append({
            "xt4": xt4,
            "xn4": xn4,
            "aw4": aw4,
            "ub": ub,
            "ones": ones,
        })

    res = run_bass_kernel_spmd(nc, in_maps, core_ids=list(range(NCORES)))
    LAST_RESULTS = res
    out = np.stack([res.results[c]["out"] for c in range(NCORES)], axis=0)
    return out.astype(np.float32)


if __name__ == "__main__":
    rng = np.random.default_rng(0)
    x = rng.standard_normal((B, N, D), dtype=np.float32)
    limit = float(np.sqrt(6.0 / (D + 2 * D)))
    W = rng.uniform(-limit, limit, size=(2 * D, D)).astype(np.float32)
    b = np.zeros((2 * D,), dtype=np.float32)
    got = kernel(x, W, b)
    print("out", got.shape, got.dtype)


# revision 21
# speedup vs baseline: 1.2879x; 1.0048x over previous
"""Trainium2 Bass kernel for single-head attention (no V projection).

Reference computation (per batch b):
    qk   = x @ W_qk.T + b_qk          # [n, 2d]
    q, k = qk[:, :d], qk[:, d:]
    dots[i, j] = k_i . q_j / sqrt(d)
    attn = softmax(dots, axis=-1)
    out[i] = sum_j attn[i, j] * x[j]

Key algebraic folding: dots[i,j] = (Wk x_i + bk).(Wq x_j + bq).  The terms
that depend only on i are constant along the softmax axis j and cancel, so
    softmax_j(dots[i,:]) == softmax_j(x_i^T A x_j + u.x_j),
with A = Wk^T Wq [d,d] and u = Wq^T bk, both input-independent and folded on
the host.  This removes the separate q/k projections: the device computes
    t   = X A + u                     # [n, d]   (stage A; u via ACT bias)
    S^T = X t^T                       # [n, n]   (stage B)
    E   = exp(S^T / 32)               # ACT, no max-subtraction (scores are
                                      #   ~N(0,0.67); exp cannot overflow)
    out = normalize(E^T X)            # (stage C; denominator via a DVE
                                      #   running sum + tiny ones-matmul)
cutting device matmul work from 3 N^2 d-class GEMMs + projections (1552
512-row matmuls) to 1296.

All matmul operands are bfloat16 (same PE rate as float32r on TRN2, half the
SBUF/DMA) — measured end-to-end error ~3e-3 scaled vs the 2e-2 gate.  Both x
layouts (xT for stages A/B, xn for stage C) and A stay SBUF-resident; there
is no DRAM spill.  Every stationary [128,128] block is loaded exactly once
(LDWEIGHTS is elided across the 2-4 matmuls sharing it and hides under the
matmul stream).

Sharding: data-parallel over batch b (8 batches -> 8 NeuronCores), no
collectives.
"""
import sys

try:
    import concourse.bass as bass  # noqa: F401
except ImportError:  # pragma: no cover
    sys.path.insert(0, "/opt/trn_rl_repo")

import numpy as np
import ml_dtypes
import concourse.bass as bass
import concourse.mybir as mybir
import concourse.tile as tile
from concourse import bacc
from concourse.bass_utils import run_bass_kernel_spmd
import concourse.bass_utils as _bu

# NOTE: walrus --enable-ldw-opt=true (used by the f32r baseline) is
# incompatible with the explicit InstLdweights that bacc synthesizes for
# multi-wait bf16 matmuls; keep the default (false).

B, N, D = 8, 2048, 1024
NCORES = 8
SCALE = 1.0 / np.sqrt(D)  # 1/32

_NC = None
LAST_RESULTS = None


def _build_nc():
    BF = mybir.dt.bfloat16
    R = mybir.dt.float32r
    F = mybir.dt.float32
    nc = bacc.Bacc("TRN2", target_bir_lowering=False, debug=False, num_devices=NCORES)

    KD = D // 128        # 8 contraction chunks over d
    NJ = N // 128        # 16 key blocks (j)
    CH = 512             # i-chunk width (one PSUM bank)
    NCH = N // CH        # 4 chunks
    NSUB = CH // 128     # 4 row-subblocks per chunk

    # Host-prepared partition-major layouts (lines of 1-2KB for DMA):
    #   xt4[p, k, i] = x[i, k*128+p];  xn4[p, j, d] = x[j*128+p, d];
    #   aw4[p, k, e] = A[k*128+p, e]
    xt4 = nc.dram_tensor("xt4", [128, KD * N], BF, kind="ExternalInput").ap()
    xn4 = nc.dram_tensor("xn4", [128, NJ * D], BF, kind="ExternalInput").ap()
    aw4 = nc.dram_tensor("aw4", [128, KD * D], BF, kind="ExternalInput").ap()
    ub = nc.dram_tensor("ub", [128, KD], F, kind="ExternalInput").ap()
    ones = nc.dram_tensor("ones", [128, 8], R, kind="ExternalInput").ap()
    out = nc.dram_tensor("out", [N, D], F, kind="ExternalOutput").ap()
    xt4r = xt4.rearrange("p (k i) -> p k i", k=KD)
    xn4r = xn4.rearrange("p (j d) -> p j d", j=NJ)
    aw4r = aw4.rearrange("p (k e) -> p k e", k=KD)

    with tile.TileContext(nc) as tc:
        with tc.tile_pool(name="pers", bufs=1) as pers, \
             tc.tile_pool(name="ob", bufs=2) as obp, \
             tc.tile_pool(name="rd", bufs=2) as rdp:

            xt_all = pers.tile([128, KD * N], BF, tag="xt", name="xt_all")
            xtr = xt_all.rearrange("p (k i) -> p k i", k=KD)
            xt = [xtr[:, k] for k in range(KD)]
            xv_all = pers.tile([128, NJ * D], BF, tag="xv", name="xv_all")
            xvr = xv_all.rearrange("p (j d) -> p j d", j=NJ)
            xv = [xvr[:, j] for j in range(NJ)]
            tT = [pers.tile([128, N], BF, tag=f"tT{k}", name=f"tT{k}")
                  for k in range(KD)]
            # f32r so the denominator ones-matmul can consume it directly
            # (the BIR verifier requires fp32r matmul inputs to be written
            # as fp32r; DVE rounds on write).
            esum = pers.tile([128, N], R, tag="esum", name="esum")
            ubt = pers.tile([128, KD], F, tag="ub", name="ubt")
            onesT = pers.tile([128, 8], R, tag="ones", name="onesT")
            warm = pers.tile([128, CH], BF, tag="warm", name="warm")
            nc.vector.memset(warm, 1.0)

            # ---------------- stage A: t^T = A^T X^T (+u) ----------------
            with tc.tile_pool(name="aw", bufs=1) as awp, \
                 tc.tile_pool(name="psA", bufs=2, space="PSUM") as psAp:
                aw_all = awp.tile([128, KD * D], BF, tag="aw", name="aw_all")
                awr = aw_all.rearrange("p (k e) -> p k e", k=KD)

                # PE p-state warm-up: dependency-free matmuls on garbage SBUF
                # fill the otherwise idle DMA head window (~7.5-14.5us: the
                # first A block's aw-m0 + xt-c0 take that long to land) and
                # keep the PE at full clock so the first real matmuls run at
                # 216ns, not at the ~630ns re-ramp rate after an idle gap.
                for w in range(20):
                    wp = psAp.tile([128, CH], F, tag="warm", name="psW")
                    nc.tensor.matmul(wp, warm[:, 0:128], warm,
                                     start=True, stop=True)

                # DMA priority order (one in-order HW queue, ~0.7us per
                # descriptor issue): exactly what the (c0, m) blocks consume,
                # in consumption order; bias early (first ACT drain needs it)
                # but after the first-matmul critical pair; stage-C tensors
                # last.
                nc.sync.dma_start(out=awr[:, :, 0:128], in_=aw4r[:, :, 0:128])
                nc.sync.dma_start(out=xtr[:, :, 0:CH], in_=xt4r[:, :, 0:CH])
                nc.sync.dma_start(out=awr[:, :, 128:256], in_=aw4r[:, :, 128:256])
                nc.sync.dma_start(out=ubt, in_=ub)
                nc.sync.dma_start(out=onesT, in_=ones)
                for m in range(2, KD):
                    nc.sync.dma_start(out=awr[:, :, m * 128:(m + 1) * 128],
                                      in_=aw4r[:, :, m * 128:(m + 1) * 128])
                for c in range(1, NCH):
                    nc.sync.dma_start(out=xtr[:, :, c * CH:(c + 1) * CH],
                                      in_=xt4r[:, :, c * CH:(c + 1) * CH])
                nc.sync.dma_start(out=xvr[:, 0:NJ // 2], in_=xn4r[:, 0:NJ // 2])
                nc.sync.dma_start(out=xvr[:, NJ // 2:NJ], in_=xn4r[:, NJ // 2:NJ])

                for c in range(NCH):
                    cols = slice(c * CH, (c + 1) * CH)
                    for m in range(KD):
                        pt = psAp.tile([128, CH], F, tag="a", name="psA")
                        for k in range(KD):
                            nc.tensor.matmul(
                                pt, awr[:, k, m * 128:(m + 1) * 128],
                                xt[k][:, cols],
                                start=(k == 0), stop=(k == KD - 1))
                        nc.scalar.activation(
                            tT[m][:, cols], pt,
                            mybir.ActivationFunctionType.Identity,
                            bias=ubt[:, m:m + 1], scale=1.0)

            with tc.tile_pool(name="e", bufs=1) as epool:
                ee = [epool.tile([128, N], BF, tag=f"e{j}", name=f"e{j}")
                      for j in range(NJ)]

                # ------------- stage B: S^T strips + exp + esum -------------
                with tc.tile_pool(name="psB", bufs=2, space="PSUM") as psBp:
                    for j in range(NJ):
                        jb = slice(j * 128, (j + 1) * 128)
                        ps = [psBp.tile([128, CH], F, tag=f"b{c}", name=f"psB{c}")
                              for c in range(NCH)]
                        for k in range(KD):
                            for c in range(NCH):
                                nc.tensor.matmul(
                                    ps[c], xt[k][:, jb],
                                    tT[k][:, c * CH:(c + 1) * CH],
                                    start=(k == 0), stop=(k == KD - 1))
                        for c in range(NCH):
                            csl = slice(c * CH, (c + 1) * CH)
                            nc.scalar.activation(
                                ee[j][:, csl], ps[c],
                                mybir.ActivationFunctionType.Exp, scale=SCALE)
                            if j == 0:
                                nc.vector.tensor_copy(esum[:, csl], ee[0][:, csl])
                            else:
                                nc.vector.tensor_add(esum[:, csl], esum[:, csl],
                                                     ee[j][:, csl])

                # ------------- stage C: out rows = attn @ X -------------
                with tc.tile_pool(name="psD", bufs=2, space="PSUM") as psDp, \
                     tc.tile_pool(name="psO", bufs=2, space="PSUM") as psOp:
                    def emit_pd(isl):
                        # denominator: pd only needs esum (ready at stage B
                        # end), so emitted before the j-loop it overlaps it
                        # and the reciprocal is off the drain path.
                        pd = psDp.tile([128, 8], F, tag="pd", name="psD")
                        nc.tensor.matmul(pd, esum[:, isl], onesT,
                                         start=True, stop=True)
                        rden = rdp.tile([128, 1], F, tag="rden", name="rden")
                        nc.vector.reciprocal(rden, pd[:, 0:1])
                        return rden

                    for idx in range(NCH * NSUB):
                        c, sub = divmod(idx, NSUB)
                        i0 = c * CH + sub * 128
                        isl = slice(i0, i0 + 128)
                        # idx 0: pd after the j-loop so C's first PE work is
                        # the j-loop (its PSUM-bank wait is the short one).
                        if idx > 0:
                            rden = emit_pd(isl)
                        p0 = psOp.tile([128, 512], F, tag="p0", name="psO0")
                        p1 = psOp.tile([128, 512], F, tag="p1", name="psO1")
                        for j in range(NJ):
                            lhs = ee[j][:, isl]
                            nc.tensor.matmul(p0, lhs, xv[j][:, 0:512],
                                             start=(j == 0), stop=(j == NJ - 1))
                            nc.tensor.matmul(p1, lhs, xv[j][:, 512:1024],
                                             start=(j == 0), stop=(j == NJ - 1))
                        if idx == 0:
                            rden = emit_pd(isl)
                        # split halves so the last transfer overlaps the
                        # second normalize (input DMAs are long done, so
                        # the sync queue is idle here).
                        ob = obp.tile([128, D], F, tag="ob", name="ob")
                        nc.vector.tensor_scalar_mul(ob[:, 0:512], p0, rden)
                        nc.sync.dma_start(out=out[i0:i0 + 128, 0:512],
                                          in_=ob[:, 0:512])
                        nc.vector.tensor_scalar_mul(ob[:, 512:1024], p1, rden)
                        nc.sync.dma_start(out=out[i0:i0 + 128, 512:1024],
                                          in_=ob[:, 512:1024])

    nc.finalize()
    return nc


def _get_nc():
    global _NC
    if _NC is None:
        _NC = _build_nc()
    return _NC


KD_HOST = D // 128
NJ_HOST = N // 128


def _prep_shared(W_qk, b_qk):
    W_qk = np.ascontiguousarray(W_qk, dtype=np.float32)
    b_qk = np.asarray(b_qk, dtype=np.float32)
    Wq, Wk = W_qk[:D], W_qk[D:]
    bk = b_qk[D:]
    A = (Wk.T @ Wq).astype(ml_dtypes.bfloat16)
    # aw4[p, k, e] = A[k*128+p, e]
    aw4 = np.ascontiguousarray(
        A.reshape(KD_HOST, 128, D).transpose(1, 0, 2).reshape(128, -1))
    u = Wq.T @ bk  # [D]; the bq/i-dependent dot terms cancel in softmax
    ub = np.ascontiguousarray(u.reshape(KD_HOST, 128).T, dtype=np.float32)
    return aw4, ub


def kernel(x: np.ndarray, W_qk: np.ndarray, b_qk: np.ndarray) -> np.ndarray:
    global LAST_RESULTS
    assert x.shape == (B, N, D), x.shape
    nc = _get_nc()

    x = np.ascontiguousarray(x, dtype=np.float32)
    aw4, ub = _prep_shared(W_qk, b_qk)
    ones = np.ones((128, 8), dtype=np.float32)
    in_maps = []
    for c in range(NCORES):
        xb = x[c].astype(ml_dtypes.bfloat16)
        # xt4[p, k, i] = x[i, k*128+p]; xn4[p, j, d] = x[j*128+p, d]
        xt4 = np.ascontiguousarray(
            xb.T.reshape(KD_HOST, 128, N).transpose(1, 0, 2).reshape(128, -1))
        xn4 = np.ascontiguousarray(
            xb.reshape(NJ_HOST, 128, D).transpose(1, 0, 2).reshape(128, -1))
        in_maps.append({
            "xt4": xt4,
            "xn4": xn4,
            "aw4": aw4,
            "ub": ub,
            "ones": ones,
        })

    res = run_bass_kernel_spmd(nc, in_maps, core_ids=list(range(NCORES)))
    LAST_RESULTS = res
    out = np.stack([res.results[c]["out"] for c in range(NCORES)], axis=0)
    return out.astype(np.float32)


if __name__ == "__main__":
    rng = np.random.default_rng(0)
    x = rng.standard_normal((B, N, D), dtype=np.float32)
    limit = float(np.sqrt(6.0 / (D + 2 * D)))
    W = rng.uniform(-limit, limit, size=(2 * D, D)).astype(np.float32)
    b = np.zeros((2 * D,), dtype=np.float32)
    got = kernel(x, W, b)
    print("out", got.shape, got.dtype)
